# revision 13
# baseline (speedup 1.0000x reference)
"""TRN2 Bass kernel for AttentionRelPos.

Problem: B=2, T=8, S=196 (14x14), DIM=768, HEADS=12, HD=64.
  qkv = x @ qkv_w.T -> q,k,v [B, 12, 1568, 64]
  attn = softmax(q k^T / 8 + decomposed rel-pos bias)
  out = (attn @ v) heads-concat @ proj_w.T + proj_b

Sharding: 24 (batch, head) pairs -> 3 per core (8 cores). Core c handles
batch c//4, heads 3*(c%4)+[0,1,2]. Each core computes a partial final
projection over its 192 channels; the host sums the 4 partials per batch
(tensor-parallel unshard) and transposes back.

Device-side trick: the decomposed rel-pos bias is folded into the QK
matmul by augmenting the contraction dim from 64 to 100:
  Q'[q] = [q/8, rel_h(q), rel_w(q), rel_t(q)]  (rel_* computed on device)
  K'[k] = [k, onehot_h(k), onehot_w(k), onehot_t(k)]
so S = Q'.K' needs no separate bias pass. Softmax denominators come from a
ones-column appended to V. All heavy matmuls run as float32r (~1.5e-4 rel
err, 4x the fp32 rate).
"""

import os
import sys

for _p in (
    "/root/.axon_site",
    "/root/.axon_site/_ro/trn_rl_repo",
    "/root/.axon_site/_ro/pypackages",
    "/opt/trn_rl_repo",
):
    if os.path.isdir(_p) and _p not in sys.path:
        sys.path.append(_p)

import numpy as np

B, T, HW_, DIM, HEADS, HD = 2, 8, 14, 768, 12, 64
S = HW_ * HW_          # 196
N = T * S              # 1568
NK = 1664              # key count padded to 13*128
KT = 13                # k tiles of 128
QC = 392               # q chunk (196-aligned, 4 per row)
NQC = 4
NAUG = 40              # 14 (h) + 14 (w) + 4 zero pad + 8 (t)
NF = HD + NAUG         # 104 = augmented contraction dim
SCALE = 0.125          # hd ** -0.5
N_CORES = 8
HEADS_PER_CORE = 3

_cached = None


def _build_bass(mm_dt_name="float32r", pt_dt_name="float32r", debug=False):
    import concourse.bass as bass
    import concourse.mybir as mybir
    import concourse.tile as tile
    from concourse import bacc

    f32 = mybir.dt.float32
    mm_dt = getattr(mybir.dt, mm_dt_name)
    pt_dt = getattr(mybir.dt, pt_dt_name)

    nc = bacc.Bacc("TRN2", target_bir_lowering=False, debug=False,
                   num_devices=N_CORES)

    d_xt = nc.dram_tensor("xt", [DIM, N], f32, kind="ExternalInput").ap()
    d_wt = nc.dram_tensor("wt", [DIM, 576], f32, kind="ExternalInput").ap()
    d_rht = nc.dram_tensor("rht", [HD, 196], f32, kind="ExternalInput").ap()
    d_rwt = nc.dram_tensor("rwt", [HD, 196], f32, kind="ExternalInput").ap()
    d_rtt = nc.dram_tensor("rtt", [HD, 64], f32, kind="ExternalInput").ap()
    d_aug = nc.dram_tensor("aug", [NAUG, NK], f32, kind="ExternalInput").ap()
    d_pwt = nc.dram_tensor("pwt", [192, DIM], f32, kind="ExternalInput").ap()
    d_pb = nc.dram_tensor("pb", [6, 128], f32, kind="ExternalInput").ap()
    d_id = nc.dram_tensor("ident", [128, 128], f32, kind="ExternalInput").ap()
    d_cns = nc.dram_tensor("cns", [128, 110], f32, kind="ExternalInput").ap()
    d_po = nc.dram_tensor("po", [DIM, N], f32, kind="ExternalOutput").ap()
    dbg = {}
    if debug:
        dbg["qt"] = nc.dram_tensor("dbg_qt", [NF, HEADS_PER_CORE, N], f32,
                                   kind="ExternalOutput").ap()
        dbg["kt"] = nc.dram_tensor("dbg_kt", [NF, HEADS_PER_CORE, NK], f32,
                                   kind="ExternalOutput").ap()
        dbg["pt"] = nc.dram_tensor("dbg_pt", [128, KT, QC], f32,
                                   kind="ExternalOutput").ap()
        dbg["ot"] = nc.dram_tensor("dbg_ot", [128, N], f32,
                                   kind="ExternalOutput").ap()

    def bc(ap):
        """View an fp32 dram AP as the matmul dtype (byte-identical load)."""
        return ap.bitcast(mm_dt) if mm_dt != f32 else ap

    with tile.TileContext(nc) as tc:
        with (
            tc.tile_pool(name="const", bufs=1) as cpool,
            tc.tile_pool(name="big", bufs=1) as bpool,
            tc.tile_pool(name="work", bufs=3) as wpool,
            tc.tile_pool(name="stage", bufs=3) as spool,
        ):
            # ---------------- constants / inputs ----------------
            xt = bpool.tile([128, 6, N], mm_dt, tag="xt")
            for c in range(6):
                nc.sync.dma_start(xt[:, c, :], bc(d_xt[c * 128:(c + 1) * 128, :]))
            wt = cpool.tile([128, 6, 576], mm_dt, tag="wt")
            for c in range(6):
                nc.sync.dma_start(wt[:, c, :], bc(d_wt[c * 128:(c + 1) * 128, :]))
            rht = cpool.tile([HD, 196], mm_dt, tag="rht")
            nc.sync.dma_start(rht[:], bc(d_rht[:]))
            rwt = cpool.tile([HD, 196], mm_dt, tag="rwt")
            nc.sync.dma_start(rwt[:], bc(d_rwt[:]))
            rtt = cpool.tile([HD, 64], mm_dt, tag="rtt")
            nc.sync.dma_start(rtt[:], bc(d_rtt[:]))
            pwt0 = cpool.tile([128, DIM], mm_dt, tag="pwt0")
            nc.sync.dma_start(pwt0[:], bc(d_pwt[0:128, :]))
            pwt1 = cpool.tile([64, DIM], mm_dt, tag="pwt1")
            nc.sync.dma_start(pwt1[:], bc(d_pwt[128:192, :]))
            pb = cpool.tile([128, 6], f32, tag="pb")
            for m in range(6):
                nc.sync.dma_start(pb[:, m:m + 1], d_pb[m, :][:, None])
            ident = cpool.tile([128, 128], mm_dt, tag="ident")
            nc.sync.dma_start(ident[:], bc(d_id[:]))

            # augmented Q'/K' tiles; rows 0:64 filled by QKV phase,
            # rows 64:100 are rel-pos (Q') / one-hot indicators (K')
            qt = bpool.tile([NF, HEADS_PER_CORE, N], mm_dt, tag="qt")
            kt_ = bpool.tile([NF, HEADS_PER_CORE, NK], mm_dt, tag="kt")
            for h in range(HEADS_PER_CORE):
                nc.sync.dma_start(kt_[HD:NF, h, :], bc(d_aug[:]))
                nc.sync.dma_start(kt_[0:HD, h, N:NK], bc(d_cns[0:HD, 0:96]))

            vt01 = bpool.tile([128, NK], mm_dt, tag="vt01")
            vt2 = bpool.tile([64, NK], mm_dt, tag="vt2")
            nc.sync.dma_start(vt01[:, N:NK], bc(d_cns[:, 0:96]))
            nc.sync.dma_start(vt2[:, N:NK], bc(d_cns[0:64, 0:96]))

            vp = [bpool.tile([128, KT, HD + 1], pt_dt, tag=f"vp{h}",
                             name=f"vp{h}")
                  for h in range(HEADS_PER_CORE)]
            bcp = (lambda ap: ap.bitcast(pt_dt)) if pt_dt != f32 else (lambda ap: ap)
            for h in range(HEADS_PER_CORE):
                # ones column (softmax denominator); zero on the padded k rows
                nc.sync.dma_start(vp[h][:, 0:KT - 1, HD:HD + 1],
                                  bcp(d_cns[:, 96:96 + KT - 1])[:, :, None])
                nc.sync.dma_start(vp[h][:, KT - 1, HD:HD + 1],
                                  bcp(d_cns[:, 108:109]))

            outT01 = bpool.tile([128, N], mm_dt, tag="outT01")
            outT2 = bpool.tile([64, N], mm_dt, tag="outT2")

            # ---------------- phase 1: QKV + rel + V-transpose ----------------
            with tc.tile_pool(name="ppsum1", bufs=2, space="PSUM") as ppsum1:
                # QKV: 5 M-tiles: [q0|k0], [q1|k1], [q2|k2], [v0|v1], [v2]
                for mt in range(5):
                    msz = 128 if mt < 4 else 64
                    for qc in range(NQC):
                        ps = ppsum1.tile([128, QC], f32, tag="qkv")
                        sl = slice(qc * QC, (qc + 1) * QC)
                        for c in range(6):
                            nc.tensor.matmul(
                                ps[0:msz, :],
                                wt[:, c, mt * 128:mt * 128 + msz],
                                xt[:, c, sl],
                                start=(c == 0), stop=(c == 5),
                            )
                        if mt < 3:
                            nc.vector.tensor_copy(qt[0:HD, mt, sl], ps[0:64, :])
                            nc.vector.tensor_copy(kt_[0:HD, mt, sl], ps[64:128, :])
                        elif mt == 3:
                            nc.vector.tensor_copy(vt01[:, sl], ps[:, :])
                        else:
                            nc.vector.tensor_copy(vt2[:, sl], ps[0:64, :])

                # rel_h / rel_w: 14 groups each, batched over heads+t-blocks
                qt5 = qt[0:HD, :, :].rearrange("p h (t i w) -> p h t i w",
                                               t=T, i=HW_, w=HW_)
                qtr_h = qt[HD:HD + 14, :, :].rearrange(
                    "p h (t i w) -> p h t i w", t=T, i=HW_, w=HW_)
                qtr_w = qt[HD + 14:HD + 28, :, :].rearrange(
                    "p h (t i w) -> p h t i w", t=T, i=HW_, w=HW_)
                for i in range(HW_):
                    ps = ppsum1.tile([14, 336], f32, tag="relhw")
                    nc.tensor.matmul(ps[:], rht[:, i * 14:(i + 1) * 14],
                                     qt5[:, :, :, i, :], start=True, stop=True)
                    src = ps[:].rearrange("p (h t w) -> p h t w", h=3, t=T)
                    nc.vector.tensor_copy(qtr_h[:, :, :, i, :], src)
                for j in range(HW_):
                    ps = ppsum1.tile([14, 336], f32, tag="relhw")
                    nc.tensor.matmul(ps[:], rwt[:, j * 14:(j + 1) * 14],
                                     qt5[:, :, :, :, j], start=True, stop=True)
                    # dst partitions 78:92 are not 32-aligned (DVE rule), so
                    # bounce through an aligned staging tile and scatter by DMA
                    rst = wpool.tile([14, 336], mm_dt, tag="rst")
                    nc.vector.tensor_copy(rst[:], ps[:])
                    nc.sync.dma_start(
                        qtr_w[:, :, :, :, j],
                        rst[:].rearrange("p (h t i) -> p h t i", h=3, t=T))
                # rel_t: per t-block, two half-blocks of 98 to keep free>=256
                qtr_t = qt[HD + 32:NF, :, :]
                for h in range(HEADS_PER_CORE):
                    nc.sync.dma_start(qt[HD + 28:HD + 32, h, :],
                                      bc(d_aug[28:32, 0:N]))
                for t in range(T):
                    for half in range(2):
                        c0 = t * S + half * 98
                        ps = ppsum1.tile([8, 294], f32, tag="relt")
                        nc.tensor.matmul(ps[:], rtt[:, t * 8:(t + 1) * 8],
                                         qt[0:HD, :, c0:c0 + 98],
                                         start=True, stop=True)
                        src = ps[:].rearrange("p (h w) -> p h w", h=3)
                        nc.vector.tensor_copy(qtr_t[:, :, c0:c0 + 98], src)

                # V transpose: vt01 [128, NK] -> per-head V' [k, 64]
                for k in range(KT):
                    sl = slice(k * 128, (k + 1) * 128)
                    ps = ppsum1.tile([128, 128], mm_dt, tag="vtr")
                    nc.tensor.transpose(ps[:], vt01[:, sl], ident[:])
                    nc.vector.tensor_copy(vp[0][:, k, 0:HD], ps[:, 0:64])
                    nc.vector.tensor_copy(vp[1][:, k, 0:HD], ps[:, 64:128])
                    ps2 = ppsum1.tile([128, 128], mm_dt, tag="vtr")
                    nc.tensor.transpose(ps2[:, 0:64], vt2[:, sl], ident[0:64, 0:64])
                    nc.vector.tensor_copy(vp[2][:, k, 0:HD], ps2[:, 0:64])

            if debug:
                for h in range(HEADS_PER_CORE):
                    nc.sync.dma_start(dbg["qt"][:, h, :], qt[:, h, :].bitcast(f32))
                    nc.sync.dma_start(dbg["kt"][:, h, :], kt_[:, h, :].bitcast(f32))

            # ---------------- phase 2: attention + projection ----------------
            with (
                tc.tile_pool(name="spsum", bufs=2, space="PSUM") as spsum,
                tc.tile_pool(name="vpsum", bufs=1, space="PSUM") as vpsum,
                tc.tile_pool(name="jpsum", bufs=1, space="PSUM") as jpsum,
                tc.tile_pool(name="ptpool", bufs=2) as ptpool,
            ):
                groups = [(0, 3), (3, 3), (6, 3), (9, 3), (12, 1)]
                for qc in range(NQC):
                    sl = slice(qc * QC, (qc + 1) * QC)
                    for h in range(HEADS_PER_CORE):
                        ptt = ptpool.tile([128, KT, QC], pt_dt, tag="pt")
                        for g0, glen in groups:
                            sp = spsum.tile([128, 3, 512], f32, tag="sp")
                            for j in range(glen):
                                k = g0 + j
                                nc.tensor.matmul(
                                    sp[:, j, 0:QC],
                                    kt_[:, h, k * 128:(k + 1) * 128],
                                    qt[:, h, sl],
                                    start=True, stop=True,
                                )
                            nc.scalar.activation(
                                ptt[:, g0:g0 + glen, :], sp[:, 0:glen, 0:QC],
                                bass.mybir.ActivationFunctionType.Exp,
                            )
                        pv = vpsum.tile([HD + 1, QC], f32, tag="pv")
                        for k in range(KT):
                            nc.tensor.matmul(pv[:], vp[h][:, k, :], ptt[:, k, :],
                                             start=(k == 0), stop=(k == KT - 1))
                        recip = wpool.tile([1, QC], f32, tag="recip")
                        nc.vector.reciprocal(recip[:], pv[HD:HD + 1, :])
                        rec64 = wpool.tile([HD, QC], f32, tag="rec64")
                        nc.gpsimd.partition_broadcast(rec64[:], recip[:])
                        dst = (outT01[h * 64:(h + 1) * 64, sl] if h < 2
                               else outT2[:, sl])
                        nc.vector.tensor_mul(dst, pv[0:HD, :], rec64[:])
                        if debug and h == 0 and qc == 0:
                            dbgpt = spool.tile([128, KT, QC], f32, tag="dbgpt")
                            nc.vector.tensor_copy(dbgpt[:], ptt[:])
                            nc.sync.dma_start(dbg["pt"][:], dbgpt[:])

                    # partial projection for this q chunk
                    for m in range(6):
                        pp = jpsum.tile([128, QC], f32, tag="pj")
                        nc.tensor.matmul(pp[:], pwt0[:, m * 128:(m + 1) * 128],
                                         outT01[:, sl], start=True, stop=False)
                        nc.tensor.matmul(pp[:], pwt1[:, m * 128:(m + 1) * 128],
                                         outT2[:, sl], start=False, stop=True)
                        st = spool.tile([128, QC], f32, tag="stage")
                        nc.scalar.activation(
                            st[:], pp[:],
                            bass.mybir.ActivationFunctionType.Identity,
                            bias=pb[:, m:m + 1],
                        )
                        nc.sync.dma_start(d_po[m * 128:(m + 1) * 128, sl], st[:])

                if debug:
                    nc.sync.dma_start(dbg["ot"][0:128, :], outT01[:].bitcast(f32))

    nc.compile()
    return nc


def _get_compiled(debug=False):
    global _cached
    key = ("dbg" if debug else "std")
    if _cached is None:
        _cached = {}
    if key not in _cached:
        mm_dt = os.environ.get("ARP_MM_DT", "float32r")
        pt_dt = os.environ.get("ARP_PT_DT", "float32r")
        _cached[key] = _build_bass(mm_dt, pt_dt, debug=debug)
    return _cached[key]


def _prepare_in_maps(x, qkv_w, proj_w, proj_b, rel_pos_h, rel_pos_w, rel_pos_t):
    x = np.asarray(x, np.float32)
    qkv_w = np.asarray(qkv_w, np.float32)
    proj_w = np.asarray(proj_w, np.float32)
    proj_b = np.asarray(proj_b, np.float32)
    rel_pos_h = np.asarray(rel_pos_h, np.float32)
    rel_pos_w = np.asarray(rel_pos_w, np.float32)
    rel_pos_t = np.asarray(rel_pos_t, np.float32)

    ii = np.arange(HW_)
    rh = 8.0 * rel_pos_h[ii[:, None] - ii[None, :] + (HW_ - 1)]  # [i, j, 64]
    rw = 8.0 * rel_pos_w[ii[:, None] - ii[None, :] + (HW_ - 1)]
    tt = np.arange(T)
    rt = 8.0 * rel_pos_t[tt[:, None] - tt[None, :] + (T - 1)]    # [t, t', 64]
    rht = np.ascontiguousarray(rh.reshape(196, HD).T)            # [64, i*14+j]
    rwt = np.ascontiguousarray(rw.reshape(196, HD).T)
    rtt = np.ascontiguousarray(rt.reshape(64, HD).T)

    aug = np.zeros((NAUG, NK), np.float32)
    k = np.arange(N)
    aug[(k // 14) % 14, k] = 1.0          # onehot_h
    aug[14 + k % 14, k] = 1.0             # onehot_w
    aug[32 + k // S, k] = 1.0             # onehot_t (rows 28:32 stay zero pad)

    xt_b = [np.ascontiguousarray(x[b].reshape(N, DIM).T) for b in range(B)]

    cns = np.zeros((128, 110), np.float32)
    cns[:, 96:108] = 1.0
    cns[0:32, 108] = 1.0

    in_maps = []
    for c in range(N_CORES):
        b = c // 4
        heads = [3 * (c % 4) + j for j in range(HEADS_PER_CORE)]
        wcols = []
        for h in heads:
            wcols.append(qkv_w[HD * h:HD * (h + 1), :] * SCALE)       # q
            wcols.append(qkv_w[DIM + HD * h:DIM + HD * (h + 1), :])   # k
        for h in heads:
            wcols.append(qkv_w[2 * DIM + HD * h:2 * DIM + HD * (h + 1), :])
        wt = np.ascontiguousarray(np.concatenate(wcols, axis=0).T)    # [768, 576]
        pcols = np.concatenate([np.arange(HD * h, HD * (h + 1)) for h in heads])
        pwt = np.ascontiguousarray(proj_w[:, pcols].T)                # [192, 768]
        in_maps.append({
            "xt": xt_b[b],
            "wt": wt,
            "rht": rht, "rwt": rwt, "rtt": rtt,
            "aug": aug,
            "pwt": pwt,
            "pb": np.ascontiguousarray(proj_b.reshape(6, 128)),
            "ident": np.eye(128, dtype=np.float32),
            "cns": cns,
        })
    return in_maps


def _unshard(results, dtype):
    out = np.zeros((B, T, S, DIM), dtype)
    for b in range(B):
        acc = results[4 * b]["po"].astype(np.float64)
        for c in range(4 * b + 1, 4 * b + 4):
            acc = acc + results[c]["po"].astype(np.float64)
        out[b] = acc.T.reshape(T, S, DIM).astype(dtype)
    return out


def kernel(x, qkv_w, proj_w, proj_b, rel_pos_h, rel_pos_w, rel_pos_t):
    from concourse import bass_utils

    debug = bool(int(os.environ.get("ARP_DEBUG", "0")))
    nc = _get_compiled(debug=debug)
    in_maps = _prepare_in_maps(x, qkv_w, proj_w, proj_b,
                               rel_pos_h, rel_pos_w, rel_pos_t)
    res = bass_utils.run_bass_kernel_spmd(nc, in_maps,
                                          core_ids=list(range(N_CORES)))
    kernel._last_results = res.results
    return _unshard(res.results, np.asarray(x).dtype)
